# revision 6
# baseline (speedup 1.0000x reference)
"""CloudCrop multi-scale kernel for 8 TRN2 NeuronCores (v2).

Sharding: 2048 (b, m) seed-rows split 8 ways -> core r owns b=r//4,
m in [(r%4)*256, (r%4)*256+256). Each core computes its 256 seeds'
full pipeline; weights replicated; no collectives.

v2 changes vs baseline:
 - All constants packed into 5 big DRAM tensors -> ~7 DMAs instead of
   88 (was ~50us of serial SP queue time).
 - F1 (conv1 feature part) stored bf16 INTERLEAVED [128, n, 2] with
   the two 128-channel halves (ct) paired per point: ap_gather with
   d=2 fetches both halves per index in one 4-byte word. 16 gathers
   instead of 32, and the 32 gpsimd fp32->bf16 copies are gone
   (conv1 matmul rhs reads the stride-2 bf16 views directly).
 - Selection fused to 3 big DVE ops, exact semantics:
     mask = (d2h < r^2) i16
     rank = scan(add, initial=-3001) i16        # count-1-3001
     sidx = mask*3000 + rank                    # valid: count-1 in
                                                # [0,1023]; invalid <0
   local_scatter(num_elems=1024) skips negative idxs; slots < 1024
   always, so no clamp op needed.
 - Height-gate penalty via Abs: pn = Relu(SLOPE*Abs(lx-c) - SLOPE*w)
   == old pn1+pn2, one DVE add instead of two.
 - BN folded into conv weights/biases (host), maxpool before bias+relu
   of conv2 (valid since relu monotone, bias per-channel) as before.
"""
import numpy as np
import ml_dtypes
import concourse.bacc as bacc
import concourse.mybir as mybir
from concourse.tile import TileContext
from concourse.bass_utils import run_bass_kernel_spmd

P = 128
B, M, N, CSEED = 2, 1024, 1024, 512
NS = 16
RADII = (0.0125, 0.025, 0.0375, 0.05)
HMIN, HMAX = -0.02, 0.04
HC, HW = (HMIN + HMAX) / 2.0, (HMAX - HMIN) / 2.0
EPS = 1e-5
SLOPE = 1e8  # height-gate ramp penalty slope
F32 = mybir.dt.float32
F32R = mybir.dt.float32r
F16 = mybir.dt.float16
F8E4 = mybir.dt.float8e4
I16 = mybir.dt.int16
U8 = mybir.dt.uint8
BF16 = mybir.dt.bfloat16
AL = mybir.AluOpType
AF = mybir.ActivationFunctionType

_CACHE = {}

# ---- packed layout (columns) ----
# fbig [128, NF] f32
FB_TAB = 0            # tab128: 1024
FB_SX = 1024          # sx128: 256
FB_RC = 1280          # rc128: 256
FB_ID = 1536          # ident: 128
FB_IOTA16 = 1664      # iota16: 16
FB_B1 = 1680          # b1 (s,ct): 8
FB_B2 = 1688          # b2 (s,ct): 8
FB_FUB = 1696         # fuseb (ot): 2
FB_TRB = 1698         # transb (ot): 2
FB_GAB = 1700         # gateb (ot): 2
NF = 1702
# wA [128, NA] bf16
WA_FEATS = 0          # feats kc*1024: 4096
WA_W1F = 4096         # w1fT (s*4+kc)*256: 4096
WA_IDB = 8192         # identb: 128
WA_W1X = 8320         # w1x9: 256
NA = 8576
# wB [128, NB] bf16
WB_W2 = 0             # w2T (s*2+kc)*256: 2048
WB_FO = 2048          # feats_own kc*256: 1024
WB_FUSE = 3072        # fuseT kc*256 (8): 2048
WB_TRANS = 5120       # transT kc*256 (4): 1024
WB_GATE = 6144        # gateT kc*256 (2): 512
NB = 6656
# xlo [4, NX] f32
XL_XYZT = 0           # xyzT: 1024
XL_LLOC = 1024        # lloc (t*3+e)*128: 768
NX = 1792


def build_graph(reps=1):
    nc = bacc.Bacc()
    dp = nc.declare_dram_parameter

    xlo = dp("xlo", [4, NX], F32, isOutput=False)
    fbig = dp("fbig", [P, NF], F32, isOutput=False)
    iota1k = dp("iota1k", [1, N], I16, isOutput=False)
    wA = dp("wA", [P, NA], BF16, isOutput=False)
    wB = dp("wB", [P, NB], BF16, isOutput=False)
    rep16 = dp("rep16", [16, P], F16, isOutput=False)
    out = dp("out", [256, 256], F32, isOutput=True)

    with TileContext(nc) as tc:
        with tc.tile_pool(name="const", bufs=1) as cn, \
             tc.tile_pool(name="f1", bufs=4) as f1p, \
             tc.tile_pool(name="sel", bufs=2) as selp, \
             tc.tile_pool(name="tiny", bufs=3) as tn, \
             tc.tile_pool(name="gx", bufs=1) as gxp, \
             tc.tile_pool(name="gath", bufs=3) as gp, \
             tc.tile_pool(name="idxp", bufs=8) as ixp, \
             tc.tile_pool(name="y1p", bufs=4) as y1pool, \
             tc.tile_pool(name="cat", bufs=18) as catp, \
             tc.tile_pool(name="chunk", bufs=2) as chp, \
             tc.tile_pool(name="psE", bufs=1, space="PSUM") as psE, \
             tc.tile_pool(name="psU", bufs=5, space="PSUM") as psU, \
             tc.tile_pool(name="psS", bufs=1, space="PSUM") as psS:

            for _rep in range(reps):
                # ---------------- packed constants to SBUF ----------------
                xlo_sb = cn.tile([4, NX], F32)
                nc.sync.dma_start(xlo_sb, xlo[:])
                fbig_sb = cn.tile([P, NF], F32)
                nc.sync.dma_start(fbig_sb, fbig[:])
                iota1k_sb = cn.tile([P, N], I16)
                nc.sync.dma_start(iota1k_sb, iota1k[:].to_broadcast([P, N]))
                wA_sb = cn.tile([P, NA], BF16)
                nc.sync.dma_start(wA_sb, wA[:])
                wB_sb = cn.tile([P, NB], BF16)
                nc.sync.dma_start(wB_sb, wB[:])
                rep16_sb = cn.tile([16, P], F16)
                nc.sync.dma_start(rep16_sb, rep16[:])

                xyzT_sb = xlo_sb[:, XL_XYZT:XL_XYZT + N]

                def lloc_v(t_, e):
                    c0 = XL_LLOC + (t_ * 3 + e) * P
                    return xlo_sb[:, c0:c0 + P]

                tab128_sb = fbig_sb[:, FB_TAB:FB_TAB + N]
                sx128_sb = fbig_sb[:, FB_SX:FB_SX + 256]
                rc128_sb = fbig_sb[:, FB_RC:FB_RC + 256]
                ident_sb = fbig_sb[:, FB_ID:FB_ID + P]
                iota16_sb = fbig_sb[:, FB_IOTA16:FB_IOTA16 + 16]

                def b1_v(s, ct):
                    c = FB_B1 + s * 2 + ct
                    return fbig_sb[:, c:c + 1]

                def b2_v(s, ct):
                    c = FB_B2 + s * 2 + ct
                    return fbig_sb[:, c:c + 1]

                fuseb_v = [fbig_sb[:, FB_FUB + o:FB_FUB + o + 1] for o in range(2)]
                transb_v = [fbig_sb[:, FB_TRB + o:FB_TRB + o + 1] for o in range(2)]
                gateb_v = [fbig_sb[:, FB_GAB + o:FB_GAB + o + 1] for o in range(2)]

                feats_v = [wA_sb[:, WA_FEATS + kc * N:WA_FEATS + (kc + 1) * N]
                           for kc in range(4)]

                def w1f_v(s, kc):
                    c = WA_W1F + (s * 4 + kc) * 256
                    return wA_sb[:, c:c + 256]

                identb_sb = wA_sb[:, WA_IDB:WA_IDB + P]
                w1x9_sb = wA_sb[:, WA_W1X:WA_W1X + 256]

                def w2_v(s, kc):
                    c = WB_W2 + (s * 2 + kc) * 256
                    return wB_sb[:, c:c + 256]

                fo_v = [wB_sb[:, WB_FO + kc * 256:WB_FO + (kc + 1) * 256]
                        for kc in range(4)]
                fuse_v = [wB_sb[:, WB_FUSE + kc * 256:WB_FUSE + (kc + 1) * 256]
                          for kc in range(8)]
                trans_v = [wB_sb[:, WB_TRANS + kc * 256:WB_TRANS + (kc + 1) * 256]
                           for kc in range(4)]
                gate_v = [wB_sb[:, WB_GATE + kc * 256:WB_GATE + (kc + 1) * 256]
                          for kc in range(2)]

                zbias = cn.tile([P, 1], F32)
                nc.vector.memset(zbias, 0.0)
                pbc = cn.tile([P, 1], F32)
                nc.vector.memset(pbc, -HC)
                pbw = cn.tile([P, 1], F32)
                nc.vector.memset(pbw, -SLOPE * HW)

                # ------------- per-tile: local frames+d2h, selection -------------
                d2h = [cn.tile([P, N], F32, tag=f"d2h{t_}", name=f"d2h{t_}")
                       for t_ in range(2)]
                idx128 = [tn.tile([P, P], I16, tag=f"idx128_{t_}", name=f"idx128_{t_}")
                          for t_ in range(2)]
                idxr = {}
                gx64 = [None, None]
                for t_ in range(2):
                    for h in range(2):
                        hs = slice(h * 512, (h + 1) * 512)
                        sqy = chp.tile([P, 512], F32, tag="sqy")
                        pn = chp.tile([P, 512], F32, tag="pn")
                        dd = d2h[t_][:, hs]
                        for e in (1, 2, 0):
                            ps_loc = psE.tile([P, 512], F32, tag="early")
                            nc.tensor.matmul(ps_loc, lhsT=lloc_v(t_, e),
                                             rhs=xyzT_sb[:, hs], start=True, stop=True)
                            if e == 1:
                                nc.scalar.activation(sqy, ps_loc, AF.Square, bias=zbias)
                            elif e == 2:
                                nc.scalar.activation(dd, ps_loc, AF.Square, bias=zbias)
                                nc.vector.tensor_add(dd, dd, sqy)
                            else:
                                # pn = Relu(SLOPE*Abs(lx - c) - SLOPE*w)
                                ab = chp.tile([P, 512], F32, tag="ab")
                                nc.scalar.activation(ab, ps_loc, AF.Abs, bias=pbc)
                                nc.scalar.activation(pn, ab, AF.Relu,
                                                     scale=SLOPE, bias=pbw)
                                nc.vector.tensor_add(dd, dd, pn)
                    for s in range(4):
                        r2 = RADII[s] * RADII[s]
                        mask = selp.tile([P, N], I16, tag="mask")
                        nc.vector.tensor_scalar(out=mask, in0=d2h[t_], scalar1=r2,
                                                scalar2=None, op0=AL.is_lt)
                        rank = selp.tile([P, N], I16, tag="rank")
                        nc.vector.tensor_tensor_scan(out=rank, data0=mask, data1=mask,
                                                     initial=-3001.0, op0=AL.add,
                                                     op1=AL.bypass)
                        sidx = selp.tile([P, N], I16, tag="sidx")
                        nc.vector.scalar_tensor_tensor(out=sidx, in0=mask,
                                                       scalar=3000.0, in1=rank,
                                                       op0=AL.mult, op1=AL.add)
                        scat = selp.tile([P, N], I16, tag="scat")
                        nc.gpsimd.local_scatter(out_ap=scat, data_ap=iota1k_sb,
                                                idxs_ap=sidx, channels=P,
                                                num_elems=N, num_idxs=N)
                        # pad invalid slots with first valid index
                        thr = tn.tile([P, 1], F32, tag="thr")
                        nc.vector.tensor_scalar(out=thr, in0=rank[:, N - 1:N],
                                                scalar1=3001.0, scalar2=None,
                                                op0=AL.add)
                        mif = tn.tile([P, 16], F32, tag="mif")
                        nc.vector.tensor_copy(mif, scat[:, 0:16])
                        validm = tn.tile([P, 16], U8, tag="validm")
                        nc.vector.tensor_scalar(out=validm, in0=iota16_sb,
                                                scalar1=thr, scalar2=None,
                                                op0=AL.is_lt)
                        padded = tn.tile([P, 16], F32, tag="padded")
                        nc.vector.tensor_copy(padded, mif[:, 0:1].to_broadcast([P, 16]))
                        nc.vector.copy_predicated(out=padded, mask=validm, data=mif)
                        # wrapped [16,128] (= padded^T) and replicated [128,128] idx
                        trps = psS.tile([16, P], F32, tag="tr", bufs=1)
                        nc.tensor.transpose(trps, padded, ident_sb)
                        trsb = tn.tile([16, P], F16, tag="trsb")
                        nc.vector.tensor_copy(trsb, trps)
                        repps = psS.tile([P, P], F32, tag="tr", bufs=1)
                        nc.tensor.matmul(repps, lhsT=rep16_sb, rhs=trsb,
                                         start=True, stop=True)
                        nc.vector.tensor_copy(idx128[t_][s * 32:(s + 1) * 32, :],
                                              repps[0:32, :])
                        ir = ixp.tile([P, P], I16, tag="idxr")
                        nc.vector.tensor_copy(ir, repps)
                        idxr[(s, t_)] = ir
                    # batched 4-scale xyz gather + P64 for this tile
                    # (2 calls of num_idxs=1024: 2048-idx gathers are ~4x
                    # slower per idx on HW)
                    g = gxp.tile([P, 2 * N], F32, tag=f"gx64_{t_}", name=f"gx64_{t_}")
                    nc.gpsimd.ap_gather(out_ap=g[:, 0:N], in_ap=tab128_sb,
                                        idxs_ap=idx128[t_][:, 0:64],
                                        channels=P, num_elems=N, d=1, num_idxs=1024)
                    nc.gpsimd.ap_gather(out_ap=g[:, N:2 * N], in_ap=tab128_sb,
                                        idxs_ap=idx128[t_][:, 64:128],
                                        channels=P, num_elems=N, d=1, num_idxs=1024)
                    gv = g.rearrange("p (m k) -> p m k", k=16)
                    sxv = sx128_sb[:, t_ * P:(t_ + 1) * P][:, :, None] \
                        .to_broadcast([P, P, 16])
                    rcv = rc128_sb[:, t_ * P:(t_ + 1) * P][:, :, None] \
                        .to_broadcast([P, P, 16])
                    nc.vector.tensor_sub(gv, gv, sxv)
                    pb = gxp.tile([P, 2 * N], BF16, tag=f"p64_{t_}", name=f"p64_{t_}")
                    nc.vector.tensor_mul(pb.rearrange("p (m k) -> p m k", k=16),
                                         gv, rcv)
                    gx64[t_] = pb

                # ------------- phase A: F1fold per scale (PE), bf16 interleaved
                # f1sb[s] layout: [128ch, 1024pt, 2ct] -> gatherable with d=2
                f1sb = []
                for s in range(4):
                    t = f1p.tile([P, 2 * N], BF16, tag="f1sb", name=f"f1sb{s}")
                    tv = t.rearrange("p (n two) -> p n two", two=2)
                    for ct in range(2):
                        for h in range(2):
                            hs = slice(h * 512, (h + 1) * 512)
                            ps_f1 = psU.tile([P, 512], F32, tag="unit")
                            for kc in range(4):
                                nc.tensor.matmul(
                                    ps_f1, lhsT=w1f_v(s, kc)[:, ct * P:(ct + 1) * P],
                                    rhs=feats_v[kc][:, hs],
                                    start=(kc == 0), stop=(kc == 3))
                            nc.scalar.activation(tv[:, hs, ct], ps_f1, AF.Copy)
                    f1sb.append(t)

                # ------------- per (t, s): gather F1 (d=2), conv1, conv2, pool
                catk = {}
                catkb = {}
                for t_ in range(2):
                    for s in range(4):
                        g2 = gp.tile([P, 2 * 2 * N], BF16, tag="f1g", name="f1g")
                        for h2 in range(2):
                            nc.gpsimd.ap_gather(
                                out_ap=g2[:, h2 * 2 * N:(h2 + 1) * 2 * N],
                                in_ap=f1sb[s],
                                idxs_ap=idxr[(s, t_)][:, h2 * 64:(h2 + 1) * 64],
                                channels=P, num_elems=N, d=2, num_idxs=1024)
                        g2v = g2.rearrange("p (h n two) -> p h n two", h=2, two=2)
                        for ch in range(4):
                            h2 = ch // 2
                            n0 = (ch % 2) * 512
                            gs = slice(ch * 512, (ch + 1) * 512)
                            y1c = y1pool.tile([P, 2 * 512], BF16, tag="y1")
                            for ct in range(2):
                                ps_y1 = psU.tile([P, 512], F32, tag="unit")
                                nc.tensor.matmul(ps_y1, lhsT=identb_sb,
                                                 rhs=g2v[:, h2, n0:n0 + 512, ct],
                                                 start=True, stop=False)
                                nc.tensor.matmul(
                                    ps_y1,
                                    lhsT=w1x9_sb[s * 32:s * 32 + 9,
                                                 ct * P:(ct + 1) * P],
                                    rhs=gx64[t_][s * 32:s * 32 + 9, gs],
                                    start=False, stop=True,
                                    tile_position=(s * 32, 0))
                                nc.scalar.activation(y1c[:, ct * 512:(ct + 1) * 512],
                                                     ps_y1, AF.Relu,
                                                     bias=b1_v(s, ct))
                            for ot in range(2):
                                key = (s, t_, ot)
                                if key not in catk:
                                    catk[key] = catp.tile([P, P], F32, tag="catk",
                                                          name=f"catk{s}{t_}{ot}")
                                ps_y2 = psU.tile([P, 512], F32, tag="unit")
                                for kc in range(2):
                                    nc.tensor.matmul(
                                        ps_y2,
                                        lhsT=w2_v(s, kc)[:, ot * P:(ot + 1) * P],
                                        rhs=y1c[:, kc * 512:(kc + 1) * 512],
                                        start=(kc == 0), stop=(kc == 1))
                                nc.vector.tensor_reduce(
                                    out=catk[key][:, ch * 32:(ch + 1) * 32],
                                    in_=ps_y2.rearrange("p (g k) -> p g k", k=16),
                                    axis=mybir.AxisListType.X, op=AL.max)
                        for ot in range(2):
                            cb = catp.tile([P, P], BF16, tag="catkb",
                                           name=f"catkb{s}{t_}{ot}")
                            nc.scalar.activation(cb, catk[(s, t_, ot)],
                                                 AF.Relu, bias=b2_v(s, ot))
                            catkb[(s, t_, ot)] = cb

                    # ---- fuse + gate + output for this m-tile ----
                    st = []
                    st_b = []
                    for ot in range(2):
                        ps_st = psS.tile([P, P], F32, tag="small")
                        for kc in range(4):
                            nc.tensor.matmul(ps_st,
                                             lhsT=trans_v[kc][:, ot * P:(ot + 1) * P],
                                             rhs=fo_v[kc][:, t_ * P:(t_ + 1) * P],
                                             start=(kc == 0), stop=(kc == 3))
                        stt_ = tn.tile([P, P], F32, tag="st")
                        nc.vector.tensor_scalar(out=stt_, in0=ps_st,
                                                scalar1=transb_v[ot],
                                                scalar2=None, op0=AL.add)
                        stb = tn.tile([P, P], BF16, tag="stb", name=f"stb{ot}")
                        nc.vector.tensor_copy(stb, stt_)
                        st.append(stt_)
                        st_b.append(stb)
                    for ot in range(2):
                        ps_g = psS.tile([P, P], F32, tag="small")
                        for kc in range(2):
                            nc.tensor.matmul(ps_g,
                                             lhsT=gate_v[kc][:, ot * P:(ot + 1) * P],
                                             rhs=st_b[kc], start=(kc == 0),
                                             stop=(kc == 1))
                        gsig = tn.tile([P, P], F32, tag="gsig")
                        nc.scalar.activation(gsig, ps_g, AF.Sigmoid,
                                             bias=gateb_v[ot])
                        ps_fu = psS.tile([P, P], F32, tag="small")
                        for kc in range(8):
                            s_, ot2 = divmod(kc, 2)
                            nc.tensor.matmul(ps_fu,
                                             lhsT=fuse_v[kc][:, ot * P:(ot + 1) * P],
                                             rhs=catkb[(s_, t_, ot2)],
                                             start=(kc == 0), stop=(kc == 7))
                        t1 = tn.tile([P, P], F32, tag="t1")
                        nc.vector.tensor_mul(t1, gsig, st[ot])
                        ob = tn.tile([P, P], F32, tag="ob")
                        nc.vector.scalar_tensor_tensor(out=ob, in0=t1,
                                                       scalar=fuseb_v[ot],
                                                       in1=ps_fu, op0=AL.add,
                                                       op1=AL.add)
                        nc.sync.dma_start(out[ot * P:(ot + 1) * P,
                                              t_ * P:(t_ + 1) * P], ob)

    nc.compile()
    return nc


def _host_prep(inputs):
    """Fold BN, transpose weights, build packed per-core arrays."""
    f32 = np.float32
    bf = ml_dtypes.bfloat16
    xyz = np.asarray(inputs["seed_xyz"], f32)
    feats = np.asarray(inputs["seed_features"], f32)
    rot = np.asarray(inputs["vp_rot"], f32)
    W1 = np.asarray(inputs["crop_W1"], f32)
    b1 = np.asarray(inputs["crop_b1"], f32)
    g1 = np.asarray(inputs["crop_g1"], f32)
    be1 = np.asarray(inputs["crop_be1"], f32)
    m1 = np.asarray(inputs["crop_m1"], f32)
    v1 = np.asarray(inputs["crop_v1"], f32)
    W2 = np.asarray(inputs["crop_W2"], f32)
    b2 = np.asarray(inputs["crop_b2"], f32)
    g2 = np.asarray(inputs["crop_g2"], f32)
    be2 = np.asarray(inputs["crop_be2"], f32)
    m2 = np.asarray(inputs["crop_m2"], f32)
    v2 = np.asarray(inputs["crop_v2"], f32)

    a1 = (g1 / np.sqrt(v1 + EPS)).astype(f32)          # (4,256)
    a2 = (g2 / np.sqrt(v2 + EPS)).astype(f32)
    b1tot = (a1 * (b1 - m1) + be1).astype(f32)
    b2tot = (a2 * (b2 - m2) + be2).astype(f32)

    w1x9 = np.zeros((P, 256), f32)
    w1fT = np.zeros((4, CSEED, 256), f32)
    w2T = np.zeros((4, 256, 256), f32)
    for s in range(4):
        W1x = a1[s][:, None] * W1[s][:, 0:3]            # (256,3)
        for e in range(3):
            for d in range(3):
                w1x9[s * 32 + e * 3 + d] = W1x[:, e]
        w1fT[s] = (a1[s][:, None] * W1[s][:, 3:]).T
        w2T[s] = (a2[s][:, None] * W2[s]).T

    fuseT = np.asarray(inputs["fuse_W"], f32).T.copy()
    transT = np.asarray(inputs["trans_W"], f32).T.copy()
    gateT = np.asarray(inputs["gate_W"], f32).T.copy()
    fuseb = np.asarray(inputs["fuse_b"], f32)
    transb = np.asarray(inputs["trans_b"], f32)
    gateb = np.asarray(inputs["gate_b"], f32)

    ident = np.eye(P, dtype=f32)
    rep16 = np.zeros((16, P), np.float16)
    for p in range(P):
        rep16[p % 16, p] = 1.0

    # shared wB (per-core only fo differs)
    wB_shared = np.zeros((P, NB), f32)
    for s in range(4):
        for kc in range(2):
            wB_shared[:, WB_W2 + (s * 2 + kc) * 256:WB_W2 + (s * 2 + kc + 1) * 256] \
                = w2T[s][kc * P:(kc + 1) * P, :]
    for kc in range(8):
        wB_shared[:, WB_FUSE + kc * 256:WB_FUSE + (kc + 1) * 256] \
            = fuseT[kc * P:(kc + 1) * P, :]
    for kc in range(4):
        wB_shared[:, WB_TRANS + kc * 256:WB_TRANS + (kc + 1) * 256] \
            = transT[kc * P:(kc + 1) * P, :]
    for kc in range(2):
        wB_shared[:, WB_GATE + kc * 256:WB_GATE + (kc + 1) * 256] \
            = gateT[kc * P:(kc + 1) * P, :]

    wA_w = np.zeros((P, NA - WA_W1F), f32)  # w1fT|identb|w1x9 part (shared)
    for s in range(4):
        for kc in range(4):
            c = (s * 4 + kc) * 256
            wA_w[:, c:c + 256] = w1fT[s][kc * P:(kc + 1) * P, :]
    wA_w[:, WA_IDB - WA_W1F:WA_IDB - WA_W1F + P] = ident
    wA_w[:, WA_W1X - WA_W1F:WA_W1X - WA_W1F + 256] = w1x9

    fbig_shared = np.zeros((P, NF), f32)
    fbig_shared[:, FB_ID:FB_ID + P] = ident
    fbig_shared[:, FB_IOTA16:FB_IOTA16 + 16] = np.arange(16, dtype=f32)[None, :]
    for s in range(4):
        for ct in range(2):
            fbig_shared[:, FB_B1 + s * 2 + ct] = b1tot[s][ct * P:(ct + 1) * P]
            fbig_shared[:, FB_B2 + s * 2 + ct] = b2tot[s][ct * P:(ct + 1) * P]
    for o in range(2):
        fbig_shared[:, FB_FUB + o] = fuseb[o * P:(o + 1) * P]
        fbig_shared[:, FB_TRB + o] = transb[o * P:(o + 1) * P]
        fbig_shared[:, FB_GAB + o] = gateb[o * P:(o + 1) * P]

    in_maps = []
    for r in range(8):
        b, q = divmod(r, 4)
        coff = q * 256
        xb = xyz[b]                                     # (1024,3)
        xo = xyz[b][coff:coff + 256]                    # (256,3) own
        ro = rot[b][coff:coff + 256]                    # (256,3,3) own

        xlo_a = np.zeros((4, NX), f32)
        xlo_a[0:3, XL_XYZT:XL_XYZT + N] = xb.T
        xlo_a[3, XL_XYZT:XL_XYZT + N] = 1.0
        for t_ in range(2):
            sl = slice(t_ * P, (t_ + 1) * P)
            for e in range(3):
                c0 = XL_LLOC + (t_ * 3 + e) * P
                xlo_a[0:3, c0:c0 + P] = ro[sl, :, e].T
                xlo_a[3, c0:c0 + P] = -(xo[sl] * ro[sl, :, e]).sum(-1)

        fbig_a = fbig_shared.copy()
        for s in range(4):
            for e in range(3):
                for d in range(3):
                    rr = s * 32 + e * 3 + d
                    fbig_a[rr, FB_TAB:FB_TAB + N] = xb[:, d]
                    fbig_a[rr, FB_SX:FB_SX + 256] = xo[:, d]
                    fbig_a[rr, FB_RC:FB_RC + 256] = ro[:, d, e] / RADII[s]

        wA_a = np.zeros((P, NA), f32)
        fb = feats[b]                                   # (512, 1024)
        for kc in range(4):
            wA_a[:, WA_FEATS + kc * N:WA_FEATS + (kc + 1) * N] \
                = fb[kc * P:(kc + 1) * P, :]
        wA_a[:, WA_W1F:] = wA_w

        wB_a = wB_shared.copy()
        for kc in range(4):
            wB_a[:, WB_FO + kc * 256:WB_FO + (kc + 1) * 256] \
                = fb[kc * P:(kc + 1) * P, coff:coff + 256]

        in_maps.append(dict(
            xlo=xlo_a, fbig=fbig_a,
            iota1k=np.arange(N, dtype=np.int16).reshape(1, N),
            wA=wA_a.astype(bf), wB=wB_a.astype(bf), rep16=rep16))
    return in_maps


def kernel(**inputs) -> np.ndarray:
    if "nc" not in _CACHE:
        _CACHE["nc"] = build_graph()
    nc = _CACHE["nc"]
    in_maps = _host_prep(inputs)
    res = run_bass_kernel_spmd(nc, in_maps, list(range(8)))
    outf = np.zeros((B, 256, M), np.float32)
    for r in range(8):
        b, q = divmod(r, 4)
        outf[b, :, q * 256:(q + 1) * 256] = res.results[r]["out"]
    return outf
